# revision 1
# baseline (speedup 1.0000x reference)
"""Multi-head causal attention (B=4,S=1024,D=768,H=12,Dh=64) on 8 trn2 cores.

Sharding: core c handles batch b=c//2 and the 6 heads hs=(c%2)*6 .. hs+6
(head-axis tensor parallel x batch parallel; 8 cores = 4 batches x 2 head-halves).

Per-core on-chip dataflow (bf16 matmul operands, fp32 PSUM accumulation):
  xT [768,1024] (host-pretransposed bf16), W{q,k,v} stacked [768, 384] bf16
  qT/kT = W-chunk.T(lhsT) @ xT    -> [64,1024] per head (transposed layout)
  v     = xT-chunk.T @ Wv          -> [1024, 6*65] per t-chunk (65th col = ones)
  scoresT[t,s] tiles = kT-chunk(lhsT) x qT(rhs); fully-causal tiles skipped,
  diag tiles masked by accumulating identity @ (-30000 strict-lower-tri) in PSUM
  exp via ScalarE Exp(scale=1/8) straight from PSUM into a flat bf16 SBUF buffer
  ctxT_aug[65, s] = sum_j v_aug_j(lhsT) @ expT_j  (row 64 = softmax denominator)
  y_aug[h, 0:65, s] DMA'd out fp32; host divides by denominators + transposes.
"""

import threading
from contextlib import ExitStack

import ml_dtypes
import numpy as np

import concourse.bass as bass
import concourse.tile as tile
from concourse import bacc, mybir
from concourse.bass_utils import run_bass_kernel_spmd

B, S, D, H, DH = 4, 1024, 768, 12, 64
NCORES = 8
HL = H // 2          # 6 local heads per core
KC = D // 128        # 6 contraction chunks
NPAIR = HL // 2      # head pairs for qk projection
F32 = mybir.dt.float32
BF16 = mybir.dt.bfloat16
MASK_VAL = -30000.0


def _attn_groups():
    """Chunk table for one head's scoresT, packed into [128,1024] PSUM groups.

    A chunk (j, c) is the scoresT tile for t-chunk j (rows j*128..j*128+128)
    and s-range [s0, s0+w) inside output half c (s in [512c, 512c+512)).
    Only causal-relevant chunks exist. `diag` chunks need the triangular mask
    added to their first 128 columns. `ps_off` is the column offset inside the
    group's PSUM tile (each chunk stays inside one 512-col PSUM bank);
    `off` is the offset in the per-head flat exp buffer.
    """
    def chunk(j, c, ps_off):
        s0 = max(512 * c, 128 * j)
        w = 512 * (c + 1) - s0
        return dict(j=j, c=c, s0=s0, w=w, diag=(s0 == 128 * j), ps_off=ps_off)

    groups = [
        [chunk(0, 1, 0), chunk(0, 0, 512)],
        [chunk(1, 1, 0), chunk(1, 0, 512), chunk(7, 1, 896)],
        [chunk(2, 1, 0), chunk(2, 0, 512), chunk(6, 1, 768)],
        [chunk(3, 1, 0), chunk(3, 0, 512), chunk(5, 1, 640)],
        [chunk(4, 1, 0)],
    ]
    base = 0
    for g in groups:
        for ch in g:
            ch["off"] = base + ch["ps_off"]
        g_w = max(ch["ps_off"] + ch["w"] for ch in g)
        base += g_w
    total = base  # 4608
    return groups, total


def _emit_kernel(ctx: ExitStack, tc: tile.TileContext, xT, wq, wk, wv, im, y):
    nc = tc.nc
    groups, exp_cols = _attn_groups()

    # identity + 0/1 causal mask arrive as a tiny host input (generating them
    # on GpSimd costs ~6us and delays the PE warm-up)
    const = ctx.enter_context(tc.tile_pool(name="const", bufs=1))
    im_sb = const.tile([128, 2, 128], BF16)
    nc.sync.dma_start(out=im_sb, in_=im[:, :, :])
    ident = im_sb[:, 0, :]
    tri01 = im_sb[:, 1, :]  # 1 where s >= t else 0

    qk_pool = ctx.enter_context(tc.tile_pool(name="qk", bufs=1))
    qT = qk_pool.tile([128, NPAIR, S], BF16)  # partitions: (h%2)*64+e, pair, s
    kT = qk_pool.tile([128, NPAIR, S], BF16)
    v_sb = qk_pool.tile([128, 8, HL * (DH + 1)], BF16)  # [t_rel, t_chunk, h*65+x]

    # pools (PSUM budget: pj 2 banks + sg 1x4 + cx 2 = 8)
    xtw = ctx.enter_context(tc.tile_pool(name="xtw", bufs=1))
    pj = ctx.enter_context(tc.tile_pool(name="pj", bufs=1, space="PSUM"))
    sg = ctx.enter_context(tc.tile_pool(name="sg", bufs=1, space="PSUM"))
    cx = ctx.enter_context(tc.tile_pool(name="cx", bufs=2, space="PSUM"))
    ex = ctx.enter_context(tc.tile_pool(name="ex", bufs=3))
    yst = ctx.enter_context(tc.tile_pool(name="yst", bufs=3))

    # PE warm-up: ~3.5us of dummy matmuls into a scratch PSUM bank so the HAM
    # clock gate opens (K=8/8, 2.4 GHz) before the real matmuls arrive.
    warm = pj.tile([128, 128], F32, tag="pjq0", name="warm")
    for i in range(28):
        nc.tensor.matmul(out=warm, lhsT=ident, rhs=tri01,
                         start=(i == 0), stop=(i == 27))

    xt = xtw.tile([128, KC, S], BF16)
    w_q = xtw.tile([128, KC, HL * DH], BF16)
    w_k = xtw.tile([128, KC, HL * DH], BF16)
    w_v = xtw.tile([128, KC, HL * DH], BF16)
    # per-chunk loads spread over four DMA queues so chunk 0 lands fast and
    # the four streams share HBM bandwidth
    for kc in range(KC):
        nc.sync.dma_start(out=xt[:, kc, :], in_=xT[kc * 128:(kc + 1) * 128, :])
        nc.scalar.dma_start(out=w_q[:, kc, :], in_=wq[kc * 128:(kc + 1) * 128, :])
        nc.scalar.dma_start(out=w_k[:, kc, :], in_=wk[kc * 128:(kc + 1) * 128, :])
        nc.gpsimd.dma_start(out=w_v[:, kc, :], in_=wv[kc * 128:(kc + 1) * 128, :])

    # ---- PE filler machinery: engines run their streams in order, so the
    # scores groups (paced by the Scalar-engine exp) must have independent
    # matmul work interleaved into the PE stream to avoid idle gaps.
    fillers = []  # list of (est_ns, emit_fn)

    def emit_fillers(budget_ns):
        while fillers and budget_ns > 0:
            est, fn = fillers.pop(0)
            fn()
            budget_ns -= est

    def proj_qk_units(pp):
        """q/k projection for pair pp as filler units (kc-outer accumulate)."""
        units = []
        for w_all, dst in ((w_q, qT), (w_k, kT)):
            pss = [pj.tile([128, 512], F32, tag=f"pjq{i}", name=f"ps{pp}{i}")
                   for i in range(2)]

            def unit(kcs, w_all=w_all, pss=pss, pp=pp, dst=dst):
                def emit():
                    for kc in kcs:
                        for i, ps in enumerate(pss):
                            nc.tensor.matmul(
                                out=ps,
                                lhsT=w_all[:, kc, pp * 128:(pp + 1) * 128],
                                rhs=xt[:, kc, i * 512:(i + 1) * 512],
                                start=(kc == 0), stop=(kc == KC - 1),
                            )
                    if kcs[-1] == KC - 1:
                        for i, ps in enumerate(pss):
                            nc.vector.tensor_copy(
                                out=dst[:, pp, i * 512:(i + 1) * 512], in_=ps)
                return emit
            units.append((900, unit([0, 1])))
            units.append((900, unit([2, 3])))
            units.append((900, unit([4, 5])))
        return units

    def proj_v_unit(j):
        def emit():
            psv = pj.tile([128, HL * DH], F32, tag=f"pjq{j % 2}", name=f"psv{j}")
            for kc in range(KC):
                nc.tensor.matmul(
                    out=psv,
                    lhsT=xt[:, kc, j * 128:(j + 1) * 128],
                    rhs=w_v[:, kc, :],
                    start=(kc == 0), stop=(kc == KC - 1),
                )
            v_dst = v_sb[:, j, :].rearrange("p (h x) -> p h x", h=HL)
            nc.vector.tensor_copy(
                out=v_dst[:, :, 0:DH],
                in_=psv.rearrange("p (h e) -> p h e", h=HL),
            )
            nc.vector.memset(v_dst[:, :, DH:DH + 1], 1.0)
        return (1100, emit)

    chunks = [ch for g in groups for ch in g]

    def ctx_unit(h, exp_pair, c):
        def emit():
            cc = sorted((ch for ch in chunks if ch["c"] == c),
                        key=lambda t: t["j"])
            pc = cx.tile([DH + 1, 512], F32, tag="cx", name=f"pc{h}{c}")
            for idx, ch in enumerate(cc):
                nc.tensor.matmul(
                    out=pc[:, ch["s0"] - 512 * c: ch["s0"] - 512 * c + ch["w"]],
                    lhsT=v_sb[:, ch["j"], :].rearrange(
                        "p (hh x) -> p hh x", hh=HL)[:, h, :],
                    rhs=exp_pair[:, h % 2, ch["off"]:ch["off"] + ch["w"]],
                    start=(idx == 0), stop=(idx == len(cc) - 1),
                )
            yt = yst.tile([DH + 1, 512], F32, tag="yst", name=f"yt{h}{c}")
            nc.vector.tensor_copy(out=yt, in_=pc)
            nc.sync.dma_start(out=y[h, :, c * 512:(c + 1) * 512], in_=yt)
        return (2200, emit)

    def scores_group(hp, g, exp_pair):
        """One scores group for both heads of pair hp into one [128,2048]
        PSUM tile (head A banks 0-1, head B banks 2-3). A/B matmuls alternate
        so their K=64 row groups (base_partition 0/64) run concurrently.
        One Exp ACT covers both heads via a strided 3D output AP. Causal
        masking of diag chunks happens afterwards on the Vector engine
        (multiply by the 0/1 triangle), keeping the PE stream pure."""
        g_w = max(ch["ps_off"] + ch["w"] for ch in g)
        ps = sg.tile([128, 2 * 1024], F32, tag="sg", name=f"sg{hp}")
        for bank in (0, 1):
            ops = [ch for ch in g if ch["ps_off"] // 512 == bank]
            for i, ch in enumerate(ops):
                first, last = (i == 0), (i == len(ops) - 1)
                for a in (0, 1):
                    half = a * 64
                    off = a * 1024 + ch["ps_off"]
                    nc.tensor.matmul(
                        out=ps[:, off:off + ch["w"]],
                        lhsT=kT[half:half + 64, hp,
                                ch["j"] * 128:(ch["j"] + 1) * 128],
                        rhs=qT[half:half + 64, hp,
                               ch["s0"]:ch["s0"] + ch["w"]],
                        start=first, stop=last,
                    )
        nc.scalar.activation(
            out=exp_pair[:, :, g[0]["off"]:g[0]["off"] + g_w],
            in_=ps.rearrange("p (h b) -> p h b", h=2)[:, :, 0:g_w],
            func=mybir.ActivationFunctionType.Exp,
            scale=1.0 / np.sqrt(DH),
        )
        for ch in g:
            if ch["diag"]:
                for a in (0, 1):
                    sl = exp_pair[:, a, ch["off"]:ch["off"] + 128]
                    nc.vector.tensor_mul(sl, sl, tri01)

    # ---- schedule ----
    for est, fn in proj_qk_units(0):
        fn()
    fillers.extend(proj_v_unit(j) for j in range(8))

    for hp in range(NPAIR):
        # queue next pair's projections; they MUST fully emit before that
        # pair's scores groups, so they are force-drained at iteration end
        proj_next = list(proj_qk_units(hp + 1)) if hp + 1 < NPAIR else []
        fillers.extend(proj_next)
        n_proj_next = len(proj_next)

        exp_pair = ex.tile([128, 2, exp_cols], BF16, tag="exp", name=f"exp{hp}")
        for gi, g in enumerate(groups):
            scores_group(hp, g, exp_pair)
            if hp == NPAIR - 1 and gi == len(groups) - 1:
                # final group: its own ctx c0 only needs earlier groups'
                # exp, so it overlaps the last Exp ACT
                for a in (0, 1):
                    _, fn = ctx_unit(2 * hp + a, exp_pair, 0)
                    fn()
            else:
                emit_fillers(2000)

        # force-drain queued proj/v units (later stages depend on them);
        # ctx units may linger as fillers for the next pair's scores
        keep = []
        for u in fillers:
            if u in proj_next or u[0] == 1100:  # proj or v units
                u[1]()
            else:
                keep.append(u)
        fillers[:] = keep

        if hp == NPAIR - 1:
            while fillers:
                est, fn = fillers.pop(0)
                fn()
            for a in (0, 1):
                _, fn = ctx_unit(2 * hp + a, exp_pair, 1)
                fn()
        else:
            for c in (0, 1):
                for a in (0, 1):
                    fillers.append(ctx_unit(2 * hp + a, exp_pair, c))


_PROGRAM = None
_PROGRAM_LOCK = threading.Lock()


def _get_program() -> bass.Bass:
    global _PROGRAM
    with _PROGRAM_LOCK:
        if _PROGRAM is None:
            nc = bacc.Bacc(None, target_bir_lowering=False)
            xT = nc.declare_dram_parameter("xT", [D, S], BF16, isOutput=False)
            wq = nc.declare_dram_parameter("wq", [D, HL * DH], BF16, isOutput=False)
            wk = nc.declare_dram_parameter("wk", [D, HL * DH], BF16, isOutput=False)
            wv = nc.declare_dram_parameter("wv", [D, HL * DH], BF16, isOutput=False)
            im = nc.declare_dram_parameter("im", [128, 2, 128], BF16, isOutput=False)
            y = nc.declare_dram_parameter("y_aug", [HL, DH + 1, S], F32, isOutput=True)
            with tile.TileContext(nc) as tc, ExitStack() as ctx:
                _emit_kernel(ctx, tc, xT, wq, wk, wv, im, y)
            nc.finalize()  # runs Bacc passes (reg alloc, wait splitting)
            _PROGRAM = nc
    return _PROGRAM


def make_in_maps(x, Wq, Wk, Wv):
    """Per-core input dicts: batch b=core//2, heads (core%2)*6..+6."""
    bf = ml_dtypes.bfloat16
    im = np.zeros((128, 2, 128), np.float32)
    im[:, 0, :] = np.eye(128)
    t = np.arange(128)
    im[:, 1, :] = (t[None, :] >= t[:, None]).astype(np.float32)
    im = im.astype(bf)
    in_maps = []
    for core in range(NCORES):
        b, hs = core // 2, (core % 2) * HL
        xTc = np.ascontiguousarray(np.asarray(x[b]).T.astype(bf))
        maps = {"xT": xTc, "im": im}
        for name, W in (("wq", Wq), ("wk", Wk), ("wv", Wv)):
            # [6,768,64] -> [768, 6*64], col = h*64+e
            maps[name] = np.ascontiguousarray(
                np.asarray(W[hs:hs + HL]).transpose(1, 0, 2)
                .reshape(D, HL * DH).astype(bf))
        in_maps.append(maps)
    return in_maps


def assemble_output(per_core_results):
    y_full = np.zeros((B, S, H * DH), np.float32)
    for core in range(NCORES):
        ya = per_core_results[core]["y_aug"]  # [6, 65, 1024]
        b, hs = core // 2, (core % 2) * HL
        ctxs = ya[:, 0:DH, :] / ya[:, DH:DH + 1, :]          # [6, 64, 1024]
        y_full[b, :, hs * DH:(hs + HL) * DH] = (
            ctxs.transpose(2, 0, 1).reshape(S, HL * DH))
    return y_full


def kernel(x, Wq, Wk, Wv):
    nc = _get_program()
    in_maps = make_in_maps(x, Wq, Wk, Wv)
    res = run_bass_kernel_spmd(nc, in_maps, core_ids=list(range(NCORES)))
    return assemble_output(res.results)



# revision 8
# speedup vs baseline: 1.2360x; 1.2360x over previous
"""Multi-head causal attention (B=4,S=1024,D=768,H=12,Dh=64) on 8 trn2 cores.

Sharding: core c handles batch b=c//2 and the 6 heads hs=(c%2)*6 .. hs+6
(head-axis tensor parallel x batch parallel; 8 cores = 4 batches x 2 head-halves).

Per-core on-chip dataflow (bf16 matmul operands, fp32 PSUM accumulation):
  xT [768,1024] (host-pretransposed bf16), W{q,k,v} packed [6kc,128,3,384] bf16
  qT/kT = W-chunk.T(lhsT) @ xT    -> [64,1024] per head (transposed layout)
  v     = xT-chunk.T @ Wv          -> [1024, 6*65] per t-chunk (65th col = ones)
  scoresT[t,s] computed in 9 "bins" of 512 cols/head, each a [128,2,512] PSUM
  tile (head A bank 0, head B bank 1), double-buffered so the ScalarE Exp of
  bin k overlaps the PE scores of bin k+1.  Only causal-relevant pieces are
  computed; the 8 diagonal pieces sit at constant stride 512 in the flat bf16
  exp buffer so causal masking is 2 strided [128,4,128] DVE multiplies per
  (pair, half).  PE idle gaps are filled with independent proj/v/ctx matmuls
  (engines run their streams in order, so the exp-paced scores chain must
  have filler work interleaved into the PE stream).
  ctxT_aug[65, s] = sum_j v_aug_j(lhsT) @ expT_j  (row 64 = softmax denom)
  y[65, h, s] staged bf16, DMA'd out; host divides by denominators+transposes.
"""

import threading
from contextlib import ExitStack

import ml_dtypes
import numpy as np

import concourse.bass as bass
import concourse.tile as tile
from concourse import bacc, mybir
from concourse.bass_utils import run_bass_kernel_spmd

B, S, D, H, DH = 4, 1024, 768, 12, 64
NCORES = 8
HL = H // 2          # 6 local heads per core
KC = D // 128        # 6 contraction chunks
NPAIR = HL // 2      # head pairs
F32 = mybir.dt.float32
BF16 = mybir.dt.bfloat16
N_WARM = 32

# ---- scores bin table -------------------------------------------------------
# A piece (j, s0, w, o) is the scoresT region for t-chunk j (psum partitions =
# t rel.), s in [s0, s0+w), placed at column o of its 512-col bin.  Bin b's
# flat exp-buffer base is 512*b.  Diagonal piece j sits at bin j offset 0, so
# the 8 causal-mask regions live at flat offsets 512*j (constant stride).
BINS = [
    [(0, 0, 512, 0)],
    [(1, 128, 384, 0), (0, 512, 128, 384)],
    [(2, 256, 256, 0), (1, 512, 256, 256)],
    [(3, 384, 128, 0), (0, 640, 384, 128)],
    [(4, 512, 512, 0)],
    [(5, 640, 384, 0), (2, 512, 128, 384)],
    [(6, 768, 256, 0), (1, 768, 256, 256)],
    [(7, 896, 128, 0), (2, 640, 384, 128)],
    [(3, 512, 512, 0)],
]
EXP_COLS = 512 * len(BINS)  # 4608


def _ctx_pieces():
    """Per output half c, pieces (j, s0, w, flat_off) to accumulate."""
    halves = {0: [], 1: []}
    for b, pieces in enumerate(BINS):
        for (j, s0, w, o) in pieces:
            halves[s0 // 512].append((j, s0, w, 512 * b + o))
    c0 = sorted(halves[0], key=lambda p: p[0])
    c1 = sorted(halves[1], key=lambda p: (p[0] == 3, p[0], p[1]))
    # j=3 piece (the only bin-8 resident) goes last so the rest of ctx c1 can
    # run while bin 8's exp is still in flight.
    return {0: c0, 1: c1}


CTX = _ctx_pieces()


def _emit_kernel(ctx: ExitStack, tc: tile.TileContext, xT, wqkv, im, y):
    nc = tc.nc

    const = ctx.enter_context(tc.tile_pool(name="const", bufs=1))
    im_sb = const.tile([128, 128], BF16)   # tri01: 1 where s_rel >= t_rel
    wsrc = const.tile([128, 128], BF16)
    nc.vector.memset(wsrc, 0.0)
    nc.sync.dma_start(out=im_sb, in_=im[:, :])

    qk_pool = ctx.enter_context(tc.tile_pool(name="qk", bufs=1))
    qT = qk_pool.tile([128, NPAIR, S], BF16)  # partitions: (h%2)*64+e
    kT = qk_pool.tile([128, NPAIR, S], BF16)
    v_sb = qk_pool.tile([128, 8, HL * (DH + 1)], BF16)
    v4 = v_sb.rearrange("p j (h x) -> p j h x", h=HL)

    xtw = ctx.enter_context(tc.tile_pool(name="xtw", bufs=1))
    xt = xtw.tile([128, KC, S], BF16)
    w_all = xtw.tile([128, KC, 3, HL * DH], BF16)

    # PSUM budget (8 banks): pj0+pj1 (proj) 2, sg 2x2 (scores, double-buffered)
    # 4, cx0+cx1 (v proj + ctx) 2.
    pj = ctx.enter_context(tc.tile_pool(name="pj", bufs=1, space="PSUM"))
    sg = ctx.enter_context(tc.tile_pool(name="sg", bufs=2, space="PSUM"))
    cx = ctx.enter_context(tc.tile_pool(name="cx", bufs=1, space="PSUM"))
    ex = ctx.enter_context(tc.tile_pool(name="ex", bufs=3))
    yst = ctx.enter_context(tc.tile_pool(name="yst", bufs=3))

    # PE warm-up from a memset tile (no DMA dependency): opens the HAM clock
    # gate while input DMAs are still landing.
    warm = pj.tile([128, 128], F32, tag="pj0", name="warm")
    for i in range(N_WARM):
        nc.tensor.matmul(out=warm, lhsT=wsrc, rhs=wsrc,
                         start=(i == 0), stop=(i == N_WARM - 1))

    # Input DMAs: per contraction chunk, spread across the three DMA-capable
    # issue queues so chunk 0 lands fast and later chunks stagger in behind
    # the proj units (issue is ~0.8us of sequencer time per dma_start).
    def xt_dma(kc):
        return lambda eng: eng.dma_start(
            out=xt[:, kc, :], in_=xT[kc * 128:(kc + 1) * 128, :])

    def w_dma(kc):
        return lambda eng: eng.dma_start(out=w_all[:, kc, :, :], in_=wqkv[kc])

    for eng, issues in (
        (nc.sync, [xt_dma(0), xt_dma(2), xt_dma(4)]),
        (nc.scalar, [w_dma(0), w_dma(2), w_dma(4), xt_dma(5)]),
        (nc.gpsimd, [w_dma(1), xt_dma(1), xt_dma(3), w_dma(3), w_dma(5)]),
    ):
        for issue in issues:
            issue(eng)
    nc.gpsimd.memset(v4[:, :, :, DH:DH + 1], 1.0)

    # ---- filler machinery: the PE stream is in-order, so the exp-paced
    # scores bins need independent matmul units interleaved between them.
    fillers = []  # list of (est_ns, kind, emit_fn)

    def emit_fillers(budget_ns):
        while fillers and budget_ns > 0:
            est, _, fn = fillers.pop(0)
            fn()
            budget_ns -= est

    def proj_qk_unit(pp, which, kcs):
        """q/k projection for pair pp, contraction chunks kcs (accumulating)."""
        dst = (qT, kT)[which]

        def emit():
            pss = [pj.tile([128, 512], F32, tag=f"pj{i}", name=f"pp{pp}{which}{i}")
                   for i in range(2)]
            for kc in kcs:
                for i, ps in enumerate(pss):
                    nc.tensor.matmul(
                        out=ps,
                        lhsT=w_all[:, kc, which, pp * 128:(pp + 1) * 128],
                        rhs=xt[:, kc, i * 512:(i + 1) * 512],
                        start=(kc == 0), stop=(kc == KC - 1))
            if kcs[-1] == KC - 1:
                for i, ps in enumerate(pss):
                    nc.vector.tensor_copy(
                        out=dst[:, pp, i * 512:(i + 1) * 512], in_=ps)
        return (900, "proj", emit)

    def proj_pair_units(pp):
        return [proj_qk_unit(pp, w, kcs)
                for w in (0, 1) for kcs in ([0, 1], [2, 3], [4, 5])]

    def v_unit(j):
        def emit():
            psv = cx.tile([128, HL * DH], F32, tag=f"cx{j % 2}", name=f"psv{j}")
            for kc in range(KC):
                nc.tensor.matmul(
                    out=psv,
                    lhsT=xt[:, kc, j * 128:(j + 1) * 128],
                    rhs=w_all[:, kc, 2, :],
                    start=(kc == 0), stop=(kc == KC - 1))
            nc.vector.tensor_copy(
                out=v4[:, j, :, 0:DH],
                in_=psv.rearrange("p (h e) -> p h e", h=HL))
        return (1000, "v", emit)

    yst_tiles = {}

    def ctx_unit(hp, c, a, exp_pair):
        def emit():
            pc = cx.tile([DH + 1, 512], F32, tag=f"cx{a}", name=f"pc{hp}{c}{a}")
            pieces = CTX[c]
            for idx, (j, s0, w, off) in enumerate(pieces):
                nc.tensor.matmul(
                    out=pc[:, s0 - 512 * c: s0 - 512 * c + w],
                    lhsT=v4[:, j, 2 * hp + a, :],
                    rhs=exp_pair[:, a, off:off + w],
                    start=(idx == 0), stop=(idx == len(pieces) - 1))
            key = (hp, c)
            if key not in yst_tiles:
                yst_tiles[key] = yst.tile([DH + 1, 2, 512], BF16, tag="yst",
                                          name=f"yt{hp}{c}")
            yt = yst_tiles[key]
            nc.vector.tensor_copy(out=yt[:, a, :], in_=pc)
            if a == 1:
                nc.sync.dma_start(
                    out=y[:, 2 * hp:2 * hp + 2, 512 * c:512 * (c + 1)], in_=yt)
        return (600 if c == 0 else 1500, "ctx", emit)

    def scores_bin(hp, b, exp_pair):
        ps = sg.tile([128, 2, 512], F32, tag="sg", name=f"sg{hp}{b}")
        pieces = BINS[b]
        for a in (0, 1):
            for idx, (j, s0, w, o) in enumerate(pieces):
                nc.tensor.matmul(
                    out=ps[:, a, o:o + w],
                    lhsT=kT[64 * a:64 * a + 64, hp, 128 * j:128 * (j + 1)],
                    rhs=qT[64 * a:64 * a + 64, hp, s0:s0 + w],
                    start=(idx == 0), stop=(idx == len(pieces) - 1))
        nc.scalar.activation(
            out=exp_pair[:, :, 512 * b:512 * (b + 1)],
            in_=ps,
            func=mybir.ActivationFunctionType.Exp,
            scale=1.0 / np.sqrt(DH))

    def emit_masks(hp, half, exp_pair):
        base = 2048 * half
        tri = im_sb[:, :].unsqueeze(1).broadcast_to([128, 4, 128])
        for a in (0, 1):
            sl = exp_pair[:, a, base:base + 2048].rearrange(
                "p (r x) -> p r x", r=4)[:, :, 0:128]
            nc.vector.tensor_mul(sl, sl, tri)

    # ---- schedule ----
    # pair-0 projections emitted directly (paced by input DMA landing);
    # one v unit covers the kT copy latency before bin 0.
    p0 = proj_pair_units(0)
    for _, _, fn in p0:
        fn()
    _, _, fn = v_unit(0)
    fn()
    fillers.extend(v_unit(j) for j in range(1, 8))

    for hp in range(NPAIR):
        if hp + 1 < NPAIR:
            fillers.extend(proj_pair_units(hp + 1))
        exp_pair = ex.tile([128, 2, EXP_COLS], BF16, tag="exp", name=f"exp{hp}")
        for b in range(len(BINS)):
            scores_bin(hp, b, exp_pair)
            if b == 3:
                emit_masks(hp, 0, exp_pair)
                for a in (0, 1):
                    fillers.append(ctx_unit(hp, 0, a, exp_pair))
            elif b == 7:
                emit_masks(hp, 1, exp_pair)
            elif b == 8:
                # ctx c1 reads bin 8's exp, so it may only be EMITTED after
                # scores_bin(hp, 8): deps come from program order, and a unit
                # emitted before its producer silently reads stale data.
                for a in (0, 1):
                    fillers.append(ctx_unit(hp, 1, a, exp_pair))
            emit_fillers(1200)
        # next pair's projections must be fully emitted before its scores
        # (their PSUM->SBUF copies feed the scores matmuls)
        keep = []
        for u in fillers:
            if u[1] in ("proj", "v"):
                u[2]()
            else:
                keep.append(u)
        fillers[:] = keep

    while fillers:
        _, _, fn = fillers.pop(0)
        fn()


_PROGRAM = None
_PROGRAM_LOCK = threading.Lock()


def _get_program() -> bass.Bass:
    global _PROGRAM
    with _PROGRAM_LOCK:
        if _PROGRAM is None:
            nc = bacc.Bacc(None, target_bir_lowering=False)
            xT = nc.declare_dram_parameter("xT", [D, S], BF16, isOutput=False)
            wqkv = nc.declare_dram_parameter("wqkv", [KC, 128, 3, HL * DH], BF16,
                                             isOutput=False)
            im = nc.declare_dram_parameter("im", [128, 128], BF16, isOutput=False)
            y = nc.declare_dram_parameter("y_aug", [DH + 1, HL, S], BF16,
                                          isOutput=True)
            with tile.TileContext(nc) as tc, ExitStack() as ctx:
                _emit_kernel(ctx, tc, xT, wqkv, im, y)
            nc.finalize()
            _PROGRAM = nc
    return _PROGRAM


def make_in_maps(x, Wq, Wk, Wv):
    """Per-core input dicts: batch b=core//2, heads (core%2)*6..+6."""
    bf = ml_dtypes.bfloat16
    t = np.arange(128)
    im = (t[None, :] >= t[:, None]).astype(bf)  # 1 where s_rel >= t_rel
    in_maps = []
    for core in range(NCORES):
        b, hs = core // 2, (core % 2) * HL
        xTc = np.ascontiguousarray(np.asarray(x[b]).T.astype(bf))
        # wqkv[kc, p, t, h*64+e] = W_t[hs+h, kc*128+p, e]
        w = np.stack([np.asarray(W[hs:hs + HL]) for W in (Wq, Wk, Wv)], axis=0)
        # w: [3, HL, D, DH] -> [KC, 128, 3, HL*DH]
        w = w.transpose(2, 0, 1, 3).reshape(KC, 128, 3, HL, DH)
        w = np.ascontiguousarray(w.reshape(KC, 128, 3, HL * DH).astype(bf))
        in_maps.append({"xT": xTc, "wqkv": w, "im": im})
    return in_maps


def assemble_output(per_core_results):
    y_full = np.zeros((B, S, H * DH), np.float32)
    for core in range(NCORES):
        ya = per_core_results[core]["y_aug"].astype(np.float32)  # [65, 6, 1024]
        b, hs = core // 2, (core % 2) * HL
        ctxs = ya[0:DH] / ya[DH:DH + 1]                  # [64, 6, 1024]
        y_full[b, :, hs * DH:(hs + HL) * DH] = (
            ctxs.transpose(2, 1, 0).reshape(S, HL * DH))
    return y_full


def kernel(x, Wq, Wk, Wv):
    nc = _get_program()
    in_maps = make_in_maps(x, Wq, Wk, Wv)
    res = run_bass_kernel_spmd(nc, in_maps, core_ids=list(range(NCORES)))
    return assemble_output(res.results)


# revision 12
# speedup vs baseline: 1.2526x; 1.0134x over previous
"""Multi-head causal attention (B=4,S=1024,D=768,H=12,Dh=64) on 8 trn2 cores.

Sharding: core c handles batch b=c//2 and the 6 heads hs=(c%2)*6 .. hs+6
(head-axis tensor parallel x batch parallel; 8 cores = 4 batches x 2 head-halves).

Per-core on-chip dataflow (bf16 matmul operands, fp32 PSUM accumulation):
  xT [768,1024] (host-pretransposed bf16), W{q,k,v} packed [6kc,128,3,384] bf16
  qT/kT = W-chunk.T(lhsT) @ xT    -> [64,1024] per head (transposed layout)
  v     = xT-chunk.T @ Wv          -> [1024, 6*65] per t-chunk (65th col = ones)
  scoresT[t,s] computed in 9 "bins" of 512 cols/head, each a [128,2,512] PSUM
  tile (head A bank 0, head B bank 1), double-buffered so the ScalarE Exp of
  bin k overlaps the PE scores of bin k+1.  Only causal-relevant pieces are
  computed; the 8 diagonal pieces sit at constant stride 512 in the flat bf16
  exp buffer so causal masking is 2 strided [128,4,128] DVE multiplies per
  (pair, half).  PE idle gaps are filled with independent proj/v/ctx matmuls
  (engines run their streams in order, so the exp-paced scores chain must
  have filler work interleaved into the PE stream).
  ctxT_aug[65, s] = sum_j v_aug_j(lhsT) @ expT_j  (row 64 = softmax denom)
  y[65, h, s] staged bf16, DMA'd out; host divides by denominators+transposes.
"""

import threading
from contextlib import ExitStack

import ml_dtypes
import numpy as np

import concourse.bass as bass
import concourse.tile as tile
from concourse import bacc, mybir
from concourse.bass_utils import run_bass_kernel_spmd

B, S, D, H, DH = 4, 1024, 768, 12, 64
NCORES = 8
HL = H // 2          # 6 local heads per core
KC = D // 128        # 6 contraction chunks
NPAIR = HL // 2      # head pairs
F32 = mybir.dt.float32
BF16 = mybir.dt.bfloat16
N_WARM = 32

# ---- scores bin table -------------------------------------------------------
# A piece (j, s0, w, o) is the scoresT region for t-chunk j (psum partitions =
# t rel.), s in [s0, s0+w), placed at column o of its 512-col bin.  Bin b's
# flat exp-buffer base is 512*b.  Chunks are packed unsplit; all 8 diagonal
# chunks land in bins 0-4 so causal masking finishes early, and bin 8 holds a
# non-diag chunk so the final ctx piece follows bin 8's exp with no mask step.
BINS = [
    [(0, 0, 512, 0)],
    [(1, 128, 384, 0), (3, 384, 128, 384)],
    [(2, 256, 256, 0), (6, 768, 256, 256)],
    [(4, 512, 512, 0)],
    [(5, 640, 384, 0), (7, 896, 128, 384)],
    [(0, 512, 512, 0)],
    [(1, 512, 512, 0)],
    [(2, 512, 512, 0)],
    [(3, 512, 512, 0)],
]
EXP_COLS = 512 * len(BINS)  # 4608

# flat offset of each diagonal chunk j (mask = its first 128 cols), and the
# bin whose emission makes the mask legal to emit: group A after bin 2,
# group B after bin 4.
DIAG_OFF = {}
DIAG_BIN = {}
for _b, _pieces in enumerate(BINS):
    for (_j, _s0, _w, _o) in _pieces:
        if _s0 == 128 * _j:
            DIAG_OFF[_j] = 512 * _b + _o
            DIAG_BIN[_j] = _b
MASK_GROUPS = {
    2: [j for j in range(8) if DIAG_BIN[j] <= 2],
    4: [j for j in range(8) if 2 < DIAG_BIN[j] <= 4],
}
assert sorted(MASK_GROUPS[2] + MASK_GROUPS[4]) == list(range(8))


def _ctx_pieces():
    """Per output half c, pieces (j, s0, w, flat_off) to accumulate.
    c1 is ordered by flat offset = bin order, so when the unit is emitted
    (after bin 8) only its final piece still waits on an exp in flight."""
    halves = {0: [], 1: []}
    for b, pieces in enumerate(BINS):
        for (j, s0, w, o) in pieces:
            halves[s0 // 512].append((j, s0, w, 512 * b + o))
    c0 = sorted(halves[0], key=lambda p: p[0])
    c1 = sorted(halves[1], key=lambda p: p[3])
    return {0: c0, 1: c1}


CTX = _ctx_pieces()


def _emit_kernel(ctx: ExitStack, tc: tile.TileContext, xT, wqkv, im, y):
    nc = tc.nc

    const = ctx.enter_context(tc.tile_pool(name="const", bufs=1))
    im_sb = const.tile([128, 128], BF16)   # tri01: 1 where s_rel >= t_rel
    wsrc = const.tile([128, 128], BF16)
    nc.vector.memset(wsrc, 0.0)
    nc.sync.dma_start(out=im_sb, in_=im[:, :])

    qk_pool = ctx.enter_context(tc.tile_pool(name="qk", bufs=1))
    qT = qk_pool.tile([128, NPAIR, S], BF16)  # partitions: (h%2)*64+e
    kT = qk_pool.tile([128, NPAIR, S], BF16)
    v_sb = qk_pool.tile([128, 8, HL * (DH + 1)], BF16)
    v4 = v_sb.rearrange("p j (h x) -> p j h x", h=HL)

    xtw = ctx.enter_context(tc.tile_pool(name="xtw", bufs=1))
    xt = xtw.tile([128, KC, S], BF16)
    w_all = xtw.tile([128, KC, 3, HL * DH], BF16)

    # PSUM budget (8 banks): pj0+pj1 (proj) 2, sg 2x2 (scores, double-buffered)
    # 4, cx0+cx1 (v proj + ctx) 2.
    pj = ctx.enter_context(tc.tile_pool(name="pj", bufs=1, space="PSUM"))
    sg = ctx.enter_context(tc.tile_pool(name="sg", bufs=2, space="PSUM"))
    cx = ctx.enter_context(tc.tile_pool(name="cx", bufs=1, space="PSUM"))
    ex = ctx.enter_context(tc.tile_pool(name="ex", bufs=3))
    yst = ctx.enter_context(tc.tile_pool(name="yst", bufs=3))

    # PE warm-up from a memset tile (no DMA dependency): opens the HAM clock
    # gate while input DMAs are still landing.
    warm = pj.tile([128, 128], F32, tag="pj0", name="warm")
    for i in range(N_WARM):
        nc.tensor.matmul(out=warm, lhsT=wsrc, rhs=wsrc,
                         start=(i == 0), stop=(i == N_WARM - 1))

    # Input DMAs: per contraction chunk, spread across the three DMA-capable
    # issue queues so chunk 0 lands fast and later chunks stagger in behind
    # the proj units (issue is ~0.8us of sequencer time per dma_start).
    def xt_dma(kc):
        return lambda eng: eng.dma_start(
            out=xt[:, kc, :], in_=xT[kc * 128:(kc + 1) * 128, :])

    def w_dma(kc):
        return lambda eng: eng.dma_start(out=w_all[:, kc, :, :], in_=wqkv[kc])

    # chunk-ascending rounds: proj unit [kc,kc+1] data lands just in time
    for eng, issues in (
        (nc.sync, [xt_dma(0), xt_dma(1), w_dma(3), w_dma(4)]),
        (nc.scalar, [w_dma(0), xt_dma(2), xt_dma(3), w_dma(5)]),
        (nc.gpsimd, [w_dma(1), w_dma(2), xt_dma(4), xt_dma(5)]),
    ):
        for issue in issues:
            issue(eng)
    nc.gpsimd.memset(v4[:, :, :, DH:DH + 1], 1.0)

    # ---- filler machinery: the PE stream is in-order, so the exp-paced
    # scores bins need independent matmul units interleaved between them.
    fillers = []  # list of (est_ns, kind, emit_fn)

    def emit_fillers(budget_ns):
        while fillers and budget_ns > 0:
            est, _, fn = fillers.pop(0)
            fn()
            budget_ns -= est

    def proj_qk_unit(pp, which, kcs):
        """q/k projection for pair pp, contraction chunks kcs (accumulating)."""
        dst = (qT, kT)[which]

        def emit():
            pss = [pj.tile([128, 512], F32, tag=f"pj{i}", name=f"pp{pp}{which}{i}")
                   for i in range(2)]
            for kc in kcs:
                for i, ps in enumerate(pss):
                    nc.tensor.matmul(
                        out=ps,
                        lhsT=w_all[:, kc, which, pp * 128:(pp + 1) * 128],
                        rhs=xt[:, kc, i * 512:(i + 1) * 512],
                        start=(kc == 0), stop=(kc == KC - 1))
            if kcs[-1] == KC - 1:
                for i, ps in enumerate(pss):
                    nc.vector.tensor_copy(
                        out=dst[:, pp, i * 512:(i + 1) * 512], in_=ps)
        return (900, "proj", emit)

    def proj_pair_units(pp):
        return [proj_qk_unit(pp, w, kcs)
                for w in (0, 1) for kcs in ([0, 1], [2, 3], [4, 5])]

    def v_unit(j):
        def emit():
            psv = cx.tile([128, HL * DH], F32, tag=f"cx{j % 2}", name=f"psv{j}")
            for kc in range(KC):
                nc.tensor.matmul(
                    out=psv,
                    lhsT=xt[:, kc, j * 128:(j + 1) * 128],
                    rhs=w_all[:, kc, 2, :],
                    start=(kc == 0), stop=(kc == KC - 1))
            nc.vector.tensor_copy(
                out=v4[:, j, :, 0:DH],
                in_=psv.rearrange("p (h e) -> p h e", h=HL))
        return (1000, "v", emit)

    yst_tiles = {}

    def ctx_unit(hp, c, a, exp_pair):
        def emit():
            pc = cx.tile([DH + 1, 512], F32, tag=f"cx{a}", name=f"pc{hp}{c}{a}")
            pieces = CTX[c]
            for idx, (j, s0, w, off) in enumerate(pieces):
                nc.tensor.matmul(
                    out=pc[:, s0 - 512 * c: s0 - 512 * c + w],
                    lhsT=v4[:, j, 2 * hp + a, :],
                    rhs=exp_pair[:, a, off:off + w],
                    start=(idx == 0), stop=(idx == len(pieces) - 1))
            key = (hp, c)
            if key not in yst_tiles:
                yst_tiles[key] = yst.tile([DH + 1, 2, 512], BF16, tag="yst",
                                          name=f"yt{hp}{c}")
            yt = yst_tiles[key]
            nc.vector.tensor_copy(out=yt[:, a, :], in_=pc)
            if a == 1:
                nc.sync.dma_start(
                    out=y[:, 2 * hp:2 * hp + 2, 512 * c:512 * (c + 1)], in_=yt)
        return (600 if c == 0 else 1500, "ctx", emit)

    def scores_bin(hp, b, exp_pair):
        ps = sg.tile([128, 2, 512], F32, tag="sg", name=f"sg{hp}{b}")
        pieces = BINS[b]
        for a in (0, 1):
            for idx, (j, s0, w, o) in enumerate(pieces):
                nc.tensor.matmul(
                    out=ps[:, a, o:o + w],
                    lhsT=kT[64 * a:64 * a + 64, hp, 128 * j:128 * (j + 1)],
                    rhs=qT[64 * a:64 * a + 64, hp, s0:s0 + w],
                    start=(idx == 0), stop=(idx == len(pieces) - 1))
        nc.scalar.activation(
            out=exp_pair[:, :, 512 * b:512 * (b + 1)],
            in_=ps,
            func=mybir.ActivationFunctionType.Exp,
            scale=1.0 / np.sqrt(DH))

    def emit_masks(hp, group, exp_pair):
        for a in (0, 1):
            for j in MASK_GROUPS[group]:
                sl = exp_pair[:, a, DIAG_OFF[j]:DIAG_OFF[j] + 128]
                nc.vector.tensor_mul(sl, sl, im_sb)

    # ---- schedule ----
    # pair-0 projections emitted directly (paced by input DMA landing);
    # one v unit covers the kT copy latency before bin 0.
    p0 = proj_pair_units(0)
    for _, _, fn in p0:
        fn()
    _, _, fn = v_unit(0)
    fn()
    fillers.extend(v_unit(j) for j in range(1, 8))

    for hp in range(NPAIR):
        if hp + 1 < NPAIR:
            fillers.extend(proj_pair_units(hp + 1))
        exp_pair = ex.tile([128, 2, EXP_COLS], BF16, tag="exp", name=f"exp{hp}")
        for b in range(len(BINS)):
            scores_bin(hp, b, exp_pair)
            if b == 2:
                emit_masks(hp, 2, exp_pair)
                for a in (0, 1):
                    fillers.append(ctx_unit(hp, 0, a, exp_pair))
            elif b == 4:
                emit_masks(hp, 4, exp_pair)
            elif b == 8:
                # ctx c1 reads bin 8's exp, so it may only be EMITTED after
                # scores_bin(hp, 8): deps come from program order, and a unit
                # emitted before its producer silently reads stale data.
                for a in (0, 1):
                    fillers.append(ctx_unit(hp, 1, a, exp_pair))
            emit_fillers(1200)
        # next pair's projections must be fully emitted before its scores
        # (their PSUM->SBUF copies feed the scores matmuls)
        keep = []
        for u in fillers:
            if u[1] in ("proj", "v"):
                u[2]()
            else:
                keep.append(u)
        fillers[:] = keep

    while fillers:
        _, _, fn = fillers.pop(0)
        fn()


_PROGRAM = None
_PROGRAM_LOCK = threading.Lock()


def _get_program() -> bass.Bass:
    global _PROGRAM
    with _PROGRAM_LOCK:
        if _PROGRAM is None:
            nc = bacc.Bacc(None, target_bir_lowering=False)
            xT = nc.declare_dram_parameter("xT", [D, S], BF16, isOutput=False)
            wqkv = nc.declare_dram_parameter("wqkv", [KC, 128, 3, HL * DH], BF16,
                                             isOutput=False)
            im = nc.declare_dram_parameter("im", [128, 128], BF16, isOutput=False)
            y = nc.declare_dram_parameter("y_aug", [DH + 1, HL, S], BF16,
                                          isOutput=True)
            with tile.TileContext(nc) as tc, ExitStack() as ctx:
                _emit_kernel(ctx, tc, xT, wqkv, im, y)
            nc.finalize()
            _PROGRAM = nc
    return _PROGRAM


def make_in_maps(x, Wq, Wk, Wv):
    """Per-core input dicts: batch b=core//2, heads (core%2)*6..+6."""
    bf = ml_dtypes.bfloat16
    t = np.arange(128)
    im = (t[None, :] >= t[:, None]).astype(bf)  # 1 where s_rel >= t_rel
    in_maps = []
    for core in range(NCORES):
        b, hs = core // 2, (core % 2) * HL
        xTc = np.ascontiguousarray(np.asarray(x[b]).T.astype(bf))
        # wqkv[kc, p, t, h*64+e] = W_t[hs+h, kc*128+p, e]
        w = np.stack([np.asarray(W[hs:hs + HL]) for W in (Wq, Wk, Wv)], axis=0)
        # w: [3, HL, D, DH] -> [KC, 128, 3, HL*DH]
        w = w.transpose(2, 0, 1, 3).reshape(KC, 128, 3, HL, DH)
        w = np.ascontiguousarray(w.reshape(KC, 128, 3, HL * DH).astype(bf))
        in_maps.append({"xT": xTc, "wqkv": w, "im": im})
    return in_maps


def assemble_output(per_core_results):
    y_full = np.zeros((B, S, H * DH), np.float32)
    for core in range(NCORES):
        ya = per_core_results[core]["y_aug"].astype(np.float32)  # [65, 6, 1024]
        b, hs = core // 2, (core % 2) * HL
        ctxs = ya[0:DH] / ya[DH:DH + 1]                  # [64, 6, 1024]
        y_full[b, :, hs * DH:(hs + HL) * DH] = (
            ctxs.transpose(2, 1, 0).reshape(S, HL * DH))
    return y_full


def kernel(x, Wq, Wk, Wv):
    nc = _get_program()
    in_maps = make_in_maps(x, Wq, Wk, Wv)
    res = run_bass_kernel_spmd(nc, in_maps, core_ids=list(range(NCORES)))
    return assemble_output(res.results)
